# revision 1
# baseline (speedup 1.0000x reference)
"""Trainium2 Bass kernel for nn_BottomUpNet (dense_mlp).

Reference computation (per row n of N=8192, fully independent across rows):
    summary = aggregate (broadcast)                   # (1024,)
    for k in 0..15:
        x = [summary, towers[n, k, :]]                # (1088,)
        h = relu(x @ OW1 + Ob1); h = relu(h @ OW2 + Ob2)
        pred_k = sigmoid(h @ OW3 + Ob3)
        m = relu(x @ MW1 + Mb1); m = relu(m @ MW2 + Mb2); m = relu(m @ MW3 + Mb3)
        summary = m
    out[n] = prod_k pred_k

Strategy: data-parallel over N across 8 cores (1024 rows each), weights
replicated.  Activations are feature-major ([feature partition, row free])
so weight matrices serve directly as the stationary matmul operand and no
on-chip transposes are needed.  Matmuls in bf16 with f32 PSUM accumulation
(end-to-end rel err vs the f32 reference ~8e-4); bias+relu epilogues on the
scalar engine out of PSUM.

Perf structure:
  - layer-1 tower matmuls (contraction 64) for the M- and O-branches are
    paired into disjoint PE row groups (0-63 / 64-127) so they stream
    concurrently in the systolic array; two m-columns are batched per
    pass so the partial-row LDWEIGHTS exposure amortizes.
  - step 0's summary is the broadcast aggregate (identical for all
    rows), so its layer-1 contribution is rank-1: computed once as
    v = agg @ W1s and injected per tile by a contraction-1 matmul
    against a ones row, replacing 256 full matmuls with ~100 cheap ones.
  - the 1024->1 output head is a DVE per-partition multiply/add tree
    (g = sum_i h2_i * w3_i) plus a single ones-vector matmul for the
    cross-partition reduce (instead of eight M=1 matmuls); its sigmoid +
    product-accumulate are deferred into the next step so they never
    head-of-line-block the scalar-engine epilogue queue.
  - the final step's M branch (M1/M2/M3) is skipped entirely: the
    reference discards the last scan carry, so that summary is dead.
  - relu epilogues alternate between the scalar and vector engines;
    weight DMAs are split across the three DGE queues strictly in
    first-use order.
"""

import numpy as np
import ml_dtypes

import concourse.bacc as bacc
import concourse.mybir as mybir
import concourse.tile as tile
from concourse.bass import ts, ds
from concourse.bass_utils import run_bass_kernel_spmd

BF16 = ml_dtypes.bfloat16

N_CORES = 8
N = 8192
K = 16
NI = 64          # tower features per step
NH = 1024        # hidden width
FT = NH // 128   # feature tiles (8)
R = N // N_CORES  # rows per core (1024)
RB = 512         # row block (matmul moving dim / one PSUM bank)
NR = R // RB     # row blocks per core (2)

_BUILT = None


def _build():
    nc = bacc.Bacc("TRN2", target_bir_lowering=False, debug=False,
                   num_devices=N_CORES)
    f32 = mybir.dt.float32
    bf = mybir.dt.bfloat16

    towd = nc.declare_dram_parameter("tow", [K, NI, R], bf, isOutput=False)
    aggd = nc.declare_dram_parameter("agg", [128, FT], f32, isOutput=False)
    mw1sd = nc.declare_dram_parameter("mw1s", [NH, NH], bf, isOutput=False)
    mw1td = nc.declare_dram_parameter("mw1t", [NI, NH], bf, isOutput=False)
    mw2d = nc.declare_dram_parameter("mw2", [NH, NH], bf, isOutput=False)
    mw3d = nc.declare_dram_parameter("mw3", [NH, NH], bf, isOutput=False)
    ow1sd = nc.declare_dram_parameter("ow1s", [NH, NH], bf, isOutput=False)
    ow1td = nc.declare_dram_parameter("ow1t", [NI, NH], bf, isOutput=False)
    ow2d = nc.declare_dram_parameter("ow2", [NH, NH], bf, isOutput=False)
    w3cd = nc.declare_dram_parameter("w3c", [128, FT], f32, isOutput=False)
    balld = nc.declare_dram_parameter("ball", [128, 40], f32, isOutput=False)
    ob3d = nc.declare_dram_parameter("ob3", [1, 1], f32, isOutput=False)
    outd = nc.declare_dram_parameter("out", [1, R], f32, isOutput=True)

    Relu = mybir.ActivationFunctionType.Relu
    Sigmoid = mybir.ActivationFunctionType.Sigmoid
    Identity = mybir.ActivationFunctionType.Identity
    Add = mybir.AluOpType.add
    Mult = mybir.AluOpType.mult

    with tile.TileContext(nc) as tc:
        with (
            tc.tile_pool(name="weights", bufs=1) as wp,
            tc.tile_pool(name="summary", bufs=1) as sp,
            tc.tile_pool(name="acts", bufs=16) as ap,
            tc.tile_pool(name="tow", bufs=4) as twp,
            tc.tile_pool(name="small", bufs=1) as smp,
            tc.tile_pool(name="zwork", bufs=2) as zw,
            tc.tile_pool(name="psum", bufs=6, space="PSUM") as pp,
            tc.tile_pool(name="zpsum", bufs=2, space="PSUM") as zp,
        ):
            # --- weights, spread across DGE queues by first use ---
            def load_w(dram, name, eng):
                tiles = []
                for i in range(FT):
                    t = wp.tile([128, NH], bf, tag=f"{name}{i}",
                                name=f"{name}{i}")
                    eng.dma_start(out=t, in_=dram[ts(i, 128), :])
                    tiles.append(t)
                return tiles

            # Step 0 runs the M branch first (layer1 unfused), so only
            # mw1s/mw1t gate the start.  Split mw1s across the two HW DGE
            # queues; everything else ordered by first use.
            def load_w_split(dram, name, engs):
                tiles = []
                for i in range(FT):
                    t = wp.tile([128, NH], bf, tag=f"{name}{i}",
                                name=f"{name}{i}")
                    engs[i % len(engs)].dma_start(out=t, in_=dram[ts(i, 128), :])
                    tiles.append(t)
                return tiles

            # The two HW DGE queues carry all big weights, strictly ordered
            # by first use (layer1 M, layer1 O, layer2, layer3) so the
            # first-needed bytes get the full HBM read bandwidth.  The
            # gpsimd SW queue carries only small early tiles and the
            # per-step tower stream.
            ball = smp.tile([128, 40], f32, tag="ball", name="ball")
            nc.gpsimd.dma_start(out=ball, in_=balld[:])
            ob3 = smp.tile([1, 1], f32, tag="ob3", name="ob3")
            nc.gpsimd.dma_start(out=ob3, in_=ob3d[:])
            aggt = smp.tile([128, FT], f32, tag="aggt", name="aggt")
            nc.gpsimd.dma_start(out=aggt, in_=aggd[:])
            w3c = smp.tile([128, FT], f32, tag="w3c", name="w3c")
            nc.gpsimd.dma_start(out=w3c, in_=w3cd[:])
            mw1t = wp.tile([NI, NH], bf, tag="mw1t", name="mw1t")
            nc.gpsimd.dma_start(out=mw1t, in_=mw1td[:])
            ow1t = wp.tile([128, NH], bf, tag="ow1t", name="ow1t")
            nc.gpsimd.memset(ow1t[64:128, :], 0.0)
            mw1s = load_w_split(mw1sd, "mw1s",
                                [nc.sync, nc.scalar, nc.gpsimd])
            ow1s = load_w_split(ow1sd, "ow1s",
                                [nc.sync, nc.scalar, nc.gpsimd])
            nc.gpsimd.dma_start(out=ow1t[64:128, :], in_=ow1td[:])
            mw2 = load_w_split(mw2d, "mw2", [nc.sync, nc.scalar])
            mw3 = load_w_split(mw3d, "mw3", [nc.sync, nc.scalar])
            ow2 = load_w_split(ow2d, "ow2", [nc.sync, nc.scalar])

            ones = smp.tile([128, 1], bf, tag="ones", name="ones")
            nc.vector.memset(ones, 1.0)
            onesrow = smp.tile([1, RB], bf, tag="onesrow", name="onesrow")
            nc.vector.memset(onesrow, 1.0)
            agg_bf = smp.tile([128, FT], bf, tag="agg_bf", name="agg_bf")
            nc.vector.tensor_copy(agg_bf[:], aggt[:])

            # --- summary double buffer.  sA is never read at k=0 (the
            # step-0 summary contribution is rank-1, see v_m/v_o below),
            # so no initialization is needed. ---
            sA = [[sp.tile([128, RB], bf, tag=f"sA{i}_{r}",
                           name=f"sA{i}_{r}") for r in range(NR)]
                  for i in range(FT)]
            sB = [[sp.tile([128, RB], bf, tag=f"sB{i}_{r}",
                           name=f"sB{i}_{r}") for r in range(NR)]
                  for i in range(FT)]

            # --- product accumulators ---
            pacc = []
            for r in range(NR):
                t = smp.tile([1, RB], f32, tag=f"pacc{r}", name=f"pacc{r}")
                nc.vector.memset(t, 1.0)
                pacc.append(t)

            # bias column index per layer: 0=Mb1 1=Mb2 2=Mb3 3=Ob1 4=Ob2
            def relu_epilogue(ot, ps, bias_l, m):
                """Bias+relu out of PSUM; alternate ACT/DVE by m so neither
                engine head-of-line-blocks the PE's psum bank rotation."""
                bias = ball[:, ds(bias_l * 8 + m, 1)]
                if m % 2 == 0:
                    nc.scalar.activation(ot[:], ps[:], Relu, bias=bias)
                else:
                    nc.vector.tensor_scalar(ot[:], ps[:], bias, 0.0, Add,
                                            mybir.AluOpType.max)

            def layer1(scur, tow_t, branches=("mo",)):
                """Fused M/O layer 1, two m-columns per batch.  All full-row
                summary matmuls for the four accumulation groups (M/O x
                m/m+1) run first; the four contraction-64 tower matmuls
                close the groups at the end, with M on PE rows 0-63 and O
                on rows 64-127 so each M/O pair streams concurrently and
                the full-row<->partial-row LDWEIGHTS exposure is amortized
                over two iterations.  fused=False (step 0) runs the M
                branch alone first so only its weights gate the start."""
                m1o = [[None] * FT for _ in range(NR)]
                h1o = [[None] * FT for _ in range(NR)]
                for br in branches:
                    for r in range(NR):
                        for mp in range(0, FT, 2):
                            psms, psos = [], []
                            for m in (mp, mp + 1):
                                if "m" in br:
                                    psm = pp.tile([128, RB],
                                                  mybir.dt.float32,
                                                  tag="ps", name="psm")
                                    psms.append(psm)
                                    for i in range(FT):
                                        nc.tensor.matmul(
                                            psm[:], mw1s[i][:, ts(m, 128)],
                                            scur[i][r][:],
                                            start=(i == 0), stop=False)
                                if "o" in br:
                                    pso = pp.tile([128, RB],
                                                  mybir.dt.float32,
                                                  tag="ps", name="pso")
                                    psos.append(pso)
                                    for i in range(FT):
                                        nc.tensor.matmul(
                                            pso[:], ow1s[i][:, ts(m, 128)],
                                            scur[i][r][:],
                                            start=(i == 0), stop=False)
                            for j, m in enumerate((mp, mp + 1)):
                                if "m" in br:
                                    nc.tensor.matmul(
                                        psms[j][:], mw1t[:, ts(m, 128)],
                                        tow_t[0:NI, ts(r, RB)],
                                        start=False, stop=True)
                                if "o" in br:
                                    nc.tensor.matmul(
                                        psos[j][:], ow1t[64:128, ts(m, 128)],
                                        tow_t[64:128, ts(r, RB)],
                                        start=False, stop=True)
                            for j, m in enumerate((mp, mp + 1)):
                                if "m" in br:
                                    m1t = ap.tile([128, RB], bf, tag="m1",
                                                  name="m1")
                                    relu_epilogue(m1t, psms[j], 0, m)
                                    m1o[r][m] = m1t
                                if "o" in br:
                                    h1t = ap.tile([128, RB], bf, tag="h1",
                                                  name="h1")
                                    relu_epilogue(h1t, psos[j], 3, m)
                                    h1o[r][m] = h1t
                reidx = lambda o: [[o[r][m] for r in range(NR)]
                                   for m in range(FT)]
                return reidx(m1o), reidx(h1o)

            def layer(rhs, ws, bias_l, out_tag, out_tiles=None):
                outs = []
                for r in range(NR):
                    row = []
                    for m in range(FT):
                        ps = pp.tile([128, RB], mybir.dt.float32, tag="ps",
                                     name="ps")
                        for i in range(FT):
                            nc.tensor.matmul(
                                ps[:], ws[i][:, ts(m, 128)], rhs[i][r][:],
                                start=(i == 0), stop=(i == FT - 1))
                        if out_tiles is not None:
                            ot = out_tiles[m][r]
                        else:
                            ot = ap.tile([128, RB], bf, tag=out_tag,
                                         name=out_tag)
                        relu_epilogue(ot, ps, bias_l, m)
                        row.append(ot)
                    outs.append(row)
                return [[outs[r][m] for r in range(NR)] for m in range(FT)]

            # Step-0 rank-1 trick: summary0 = broadcast(aggregate) is the
            # same for every row, so its layer-1 contribution is a single
            # vector v = agg @ W1s per branch, computed once with 16 small
            # matmuls and injected per tile by a contraction-1 broadcast
            # matmul against a ones row.
            def compute_v(W, name):
                v_sb = smp.tile([1, NH], bf, tag=name, name=name)
                for half in range(2):
                    vp = pp.tile([1, RB], mybir.dt.float32, tag="ps",
                                 name="vps")
                    for i in range(FT):
                        nc.tensor.matmul(
                            vp[:], agg_bf[:, ds(i, 1)],
                            W[i][:, ts(half, RB)],
                            start=(i == 0), stop=(i == FT - 1))
                    nc.scalar.activation(
                        v_sb[0:1, ts(half, RB)], vp[:],
                        mybir.ActivationFunctionType.Identity)
                return v_sb

            def layer1_k0(tow_t, v_m, v_o):
                m1o = [[None] * FT for _ in range(NR)]
                h1o = [[None] * FT for _ in range(NR)]
                for br in ("m", "o"):
                    v_sb = v_m if br == "m" else v_o
                    for r in range(NR):
                        for m in range(FT):
                            ps = pp.tile([128, RB], mybir.dt.float32,
                                         tag="ps", name="psk0")
                            nc.tensor.matmul(
                                ps[:], v_sb[0:1, ts(m, 128)], onesrow[:],
                                start=True, stop=False)
                            if br == "m":
                                nc.tensor.matmul(
                                    ps[:], mw1t[:, ts(m, 128)],
                                    tow_t[0:NI, ts(r, RB)],
                                    start=False, stop=True)
                                ot = ap.tile([128, RB], bf, tag="m1",
                                             name="m1")
                                relu_epilogue(ot, ps, 0, m)
                                m1o[r][m] = ot
                            else:
                                nc.tensor.matmul(
                                    ps[:], ow1t[64:128, ts(m, 128)],
                                    tow_t[64:128, ts(r, RB)],
                                    start=False, stop=True)
                                ot = ap.tile([128, RB], bf, tag="h1",
                                             name="h1")
                                relu_epilogue(ot, ps, 3, m)
                                h1o[r][m] = ot
                reidx = lambda o: [[o[r][m] for r in range(NR)]
                                   for m in range(FT)]
                return reidx(m1o), reidx(h1o)

            def flush_zjobs(zjobs):
                for gb, r in zjobs:
                    zps = zp.tile([1, RB], mybir.dt.float32, tag="z",
                                  name="zps")
                    nc.tensor.matmul(zps[:], ones[:], gb[:],
                                     start=True, stop=True)
                    pr = smp.tile([1, RB], mybir.dt.float32, tag=f"pr{r}",
                                  name=f"pr{r}")
                    nc.scalar.activation(pr[:], zps[:], Sigmoid, bias=ob3[:])
                    nc.vector.tensor_mul(pacc[r][:], pacc[r][:], pr[:])

            scur, snxt = sA, sB
            zjobs = []
            for k in range(K):
                tow_t = twp.tile([128, R], bf, tag="tow", name="tow")
                nc.gpsimd.dma_start(out=tow_t[0:NI, :], in_=towd[k])
                nc.gpsimd.dma_start(out=tow_t[64:128, :], in_=towd[k])

                if k == 0:
                    v_m = compute_v(mw1s, "v_m")
                    v_o = compute_v(ow1s, "v_o")
                    m1, h1 = layer1_k0(tow_t, v_m, v_o)
                elif k == K - 1:
                    # the final scan carry is discarded by the reference, so
                    # the last step's M branch (M1/M2/M3) is dead code
                    m1, h1 = layer1(scur, tow_t, branches=("o",))
                else:
                    m1, h1 = layer1(scur, tow_t)
                if k < K - 1:
                    m2 = layer(m1, mw2, 1, "l2")
                    # previous step's output head (its DVE reduce is long
                    # done, so the sigmoid never head-of-line-blocks the
                    # ACT queue)
                    flush_zjobs(zjobs)
                    zjobs = []
                    layer(m2, mw3, 2, None, out_tiles=snxt)
                else:
                    flush_zjobs(zjobs)
                    zjobs = []
                h2 = layer(h1, ow2, 4, "l2")
                # g = sum_i h2_i * w3_i on the DVE (per-partition scalars),
                # reduced across partitions next step by a ones-matmul.
                for r in range(NR):
                    if k < K - 1:
                        # DVE-serial chain; latency is hidden by the next
                        # step's PE work
                        g = zw.tile([128, RB], mybir.dt.float32, tag="g",
                                    name="g")
                        nc.vector.tensor_scalar(
                            g[:], h2[0][r][:], w3c[:, ds(0, 1)], None, Mult)
                        for i in range(1, FT):
                            t = zw.tile([128, RB], mybir.dt.float32,
                                        tag="t", name="t", bufs=3)
                            nc.vector.tensor_scalar(
                                t[:], h2[i][r][:], w3c[:, ds(i, 1)], None,
                                Mult)
                            nc.vector.tensor_tensor(g[:], g[:], t[:], Add)
                    else:
                        # final step: the chain is on the kernel's critical
                        # tail, so pipeline the multiplies on the otherwise
                        # idle scalar engine against the DVE adds
                        tts = []
                        g = None
                        for i in range(FT):
                            t = zw.tile([128, RB], mybir.dt.float32,
                                        tag="t", name="t", bufs=3)
                            nc.scalar.activation(t[:], h2[i][r][:],
                                                 Identity,
                                                 scale=w3c[:, ds(i, 1)])
                            tts.append(t)
                            if i == 1:
                                g = zw.tile([128, RB], mybir.dt.float32,
                                            tag="g", name="g")
                                nc.vector.tensor_tensor(
                                    g[:], tts[0][:], tts[1][:], Add)
                            elif i >= 2:
                                nc.vector.tensor_tensor(g[:], g[:], t[:],
                                                        Add)
                    gb = zw.tile([128, RB], bf, tag="gb", name="gb", bufs=4)
                    nc.vector.tensor_copy(gb[:], g[:])
                    zjobs.append((gb, r))

                scur, snxt = snxt, scur
            flush_zjobs(zjobs)

            for r in range(NR):
                nc.sync.dma_start(out=outd[:, ts(r, RB)], in_=pacc[r][:])

    nc.finalize()
    return nc


def _get_nc():
    global _BUILT
    if _BUILT is None:
        _BUILT = _build()
    return _BUILT


def _prep_inputs(inputs):
    f32 = np.float32
    towers = np.asarray(inputs["towers"], dtype=f32)
    agg = np.asarray(inputs["aggregate"], dtype=f32)
    MW1 = np.asarray(inputs["MW1"], dtype=f32)
    OW1 = np.asarray(inputs["OW1"], dtype=f32)

    shared = {
        "agg": np.ascontiguousarray(agg.reshape(FT, 128).T),
        "mw1s": MW1[:NH].astype(BF16),
        "mw1t": np.ascontiguousarray(MW1[NH:]).astype(BF16),
        "mw2": np.asarray(inputs["MW2"], f32).astype(BF16),
        "mw3": np.asarray(inputs["MW3"], f32).astype(BF16),
        "ow1s": OW1[:NH].astype(BF16),
        "ow1t": np.ascontiguousarray(OW1[NH:]).astype(BF16),
        "ow2": np.asarray(inputs["OW2"], f32).astype(BF16),
        "w3c": np.ascontiguousarray(
            np.asarray(inputs["OW3"], f32).reshape(FT, 128).T),
        "ball": np.concatenate(
            [np.asarray(inputs[b], f32).reshape(FT, 128).T
             for b in ("Mb1", "Mb2", "Mb3", "Ob1", "Ob2")], axis=1),
        "ob3": np.asarray(inputs["Ob3"], f32).reshape(1, 1),
    }
    in_maps = []
    for c in range(N_CORES):
        tc_ = towers[c * R:(c + 1) * R]          # (R, K, NI)
        towT = np.ascontiguousarray(tc_.transpose(1, 2, 0)).astype(BF16)
        in_maps.append({"tow": towT, **shared})
    return in_maps


def _run(inputs, trace=False):
    nc = _get_nc()
    in_maps = _prep_inputs(inputs)
    res = run_bass_kernel_spmd(nc, in_maps, list(range(N_CORES)), trace=trace)
    out = np.concatenate([res.results[c]["out"][0] for c in range(N_CORES)])
    return out.astype(np.float32), res


def kernel(**inputs):
    out, _ = _run(inputs, trace=False)
    return out



# revision 3
# speedup vs baseline: 1.8297x; 1.8297x over previous
"""Trainium2 Bass kernel for nn_BottomUpNet (dense_mlp).

Reference computation (per row n of N=8192, fully independent across rows):
    summary = aggregate (broadcast)                   # (1024,)
    for k in 0..15:
        x = [summary, towers[n, k, :]]                # (1088,)
        h = relu(x @ OW1 + Ob1); h = relu(h @ OW2 + Ob2)
        pred_k = sigmoid(h @ OW3 + Ob3)
        m = relu(x @ MW1 + Mb1); m = relu(m @ MW2 + Mb2); m = relu(m @ MW3 + Mb3)
        summary = m
    out[n] = prod_k pred_k

Strategy: data-parallel over N across 8 cores (1024 rows each), weights
replicated.  Activations are feature-major ([feature partition, row free])
so weight matrices serve directly as the stationary matmul operand and no
on-chip transposes are needed.

The five 1024-contraction matmuls per step (M1s, M2, M3, O1s, O2) run in
fp8-e4m3 with perf_mode=DoubleRow: both operands carry contraction pairs
[128, 2, free] so each matmul instruction reduces 256 rows (2 fp8 weights
per PE cell), ~1.7x the bf16 streaming rate.  Weights are pre-interleaved
on the host into [ktile, 128, 2, NH]; activations feeding these matmuls
are written by the epilogues as fp8 pair-tiles [128, 2, 512].  End-to-end
rel err vs the f32 reference ~7e-3 (fp8 quantization noise; the e4m3
denormal range covers the small uniform weights acceptably, so no weight
scaling is needed and bias+relu epilogues keep their single-op form).
f32 PSUM accumulation throughout; the 64-wide tower matmuls and the
output head stay bf16/f32.

Perf structure:
  - loop order is m-outer / row-block-inner so each DoubleRow stationary
    tile (256x128 weight block) is reused by NR=2 matmuls, halving
    LDWEIGHTS traffic (DoubleRow weight loads are 2x the columns).
  - layer-1 tower closers for the M- and O-branches are paired into
    disjoint PE row groups (0-63 / 64-127), issued adjacently so each
    M/O pair streams concurrently in the systolic array.
  - step 0's summary is the broadcast aggregate, identical for all rows:
    its layer-1 contribution agg @ W1s is folded into the step-0 bias on
    the host, so step 0's layer 1 is just the tower matmul.
  - the 1024->1 output head is a DVE per-partition multiply/add tree
    (g = sum_i h2_i * w3_i) plus a single ones-vector matmul for the
    cross-partition reduce; its sigmoid + product-accumulate are deferred
    into the next step so they never head-of-line-block the scalar queue.
  - the final step's M branch (M1/M2/M3) is skipped entirely: the
    reference discards the last scan carry, so that summary is dead.
  - relu epilogues alternate between the scalar and vector engines;
    weight DMAs are split across the DGE queues strictly in first-use
    order (step 0 needs only the tower weights + biases to start).
"""

import numpy as np
import ml_dtypes

import concourse.bacc as bacc
import concourse.mybir as mybir
import concourse.tile as tile
from concourse.bass import ts, ds
from concourse.bass_utils import run_bass_kernel_spmd

BF16 = ml_dtypes.bfloat16
F8 = ml_dtypes.float8_e4m3

N_CORES = 8
N = 8192
K = 16
NI = 64          # tower features per step
NH = 1024        # hidden width
FT = NH // 128   # feature tiles (8)
KT = NH // 256   # DoubleRow contraction tiles (4)
R = N // N_CORES  # rows per core (1024)
RB = 512         # row block (matmul moving dim / one PSUM bank)
NR = R // RB     # row blocks per core (2)

_BUILT = None


def _build():
    nc = bacc.Bacc("TRN2", target_bir_lowering=False, debug=False,
                   num_devices=N_CORES)
    f32 = mybir.dt.float32
    bf = mybir.dt.bfloat16
    f8 = mybir.dt.float8e4
    DR = mybir.MatmulPerfMode.DoubleRow

    towd = nc.declare_dram_parameter("tow", [K, NI, R], bf, isOutput=False)
    mw1sd = nc.declare_dram_parameter("mw1s", [KT, 128, 2, NH], f8,
                                      isOutput=False)
    mw1td = nc.declare_dram_parameter("mw1t", [NI, NH], bf, isOutput=False)
    mw2d = nc.declare_dram_parameter("mw2", [KT, 128, 2, NH], f8,
                                     isOutput=False)
    mw3d = nc.declare_dram_parameter("mw3", [KT, 128, 2, NH], f8,
                                     isOutput=False)
    ow1sd = nc.declare_dram_parameter("ow1s", [KT, 128, 2, NH], f8,
                                      isOutput=False)
    ow1td = nc.declare_dram_parameter("ow1t", [NI, NH], bf, isOutput=False)
    ow2d = nc.declare_dram_parameter("ow2", [KT, 128, 2, NH], f8,
                                     isOutput=False)
    w3cd = nc.declare_dram_parameter("w3c", [128, FT], f32, isOutput=False)
    balld = nc.declare_dram_parameter("ball", [128, 56], f32, isOutput=False)
    ob3d = nc.declare_dram_parameter("ob3", [1, 1], f32, isOutput=False)
    outd = nc.declare_dram_parameter("out", [1, R], f32, isOutput=True)

    Relu = mybir.ActivationFunctionType.Relu
    Sigmoid = mybir.ActivationFunctionType.Sigmoid
    Identity = mybir.ActivationFunctionType.Identity
    Add = mybir.AluOpType.add
    Mult = mybir.AluOpType.mult

    with tile.TileContext(nc) as tc:
        with (
            tc.tile_pool(name="weights", bufs=1) as wp,
            tc.tile_pool(name="summary", bufs=1) as sp,
            tc.tile_pool(name="acts", bufs=16) as ap,
            tc.tile_pool(name="tow", bufs=4) as twp,
            tc.tile_pool(name="small", bufs=1) as smp,
            tc.tile_pool(name="zwork", bufs=2) as zw,
            tc.tile_pool(name="psum", bufs=6, space="PSUM") as pp,
            tc.tile_pool(name="zpsum", bufs=2, space="PSUM") as zp,
        ):
            # --- small/early tiles on the gpsimd SW queue; step 0 only
            # needs the tower weights + biases to start ---
            ball = smp.tile([128, 56], f32, tag="ball", name="ball")
            nc.gpsimd.dma_start(out=ball, in_=balld[:])
            ob3 = smp.tile([1, 1], f32, tag="ob3", name="ob3")
            nc.gpsimd.dma_start(out=ob3, in_=ob3d[:])
            w3c = smp.tile([128, FT], f32, tag="w3c", name="w3c")
            nc.gpsimd.dma_start(out=w3c, in_=w3cd[:])
            mw1t = wp.tile([NI, NH], bf, tag="mw1t", name="mw1t")
            nc.gpsimd.dma_start(out=mw1t, in_=mw1td[:])
            ow1t = wp.tile([128, NH], bf, tag="ow1t", name="ow1t")
            nc.gpsimd.dma_start(out=ow1t[64:128, :], in_=ow1td[:])

            # --- DoubleRow weights on the two HW DGE queues, strictly in
            # first-use order (step 0: M2, M3, O2; step 1 adds M1s, O1s) ---
            _q = [0]

            def load_dr(dram, name):
                tiles = []
                for j in range(KT):
                    t = wp.tile([128, 2, NH], f8, tag=f"{name}{j}",
                                name=f"{name}{j}")
                    eng = (nc.sync, nc.scalar)[_q[0] % 2]
                    _q[0] += 1
                    eng.dma_start(out=t, in_=dram[j])
                    tiles.append(t)
                return tiles

            mw2 = load_dr(mw2d, "mw2")
            mw3 = load_dr(mw3d, "mw3")
            ow2 = load_dr(ow2d, "ow2")
            mw1s = load_dr(mw1sd, "mw1s")
            ow1s = load_dr(ow1sd, "ow1s")

            ones = smp.tile([128, 1], bf, tag="ones", name="ones")
            nc.vector.memset(ones, 1.0)

            # --- summary double buffer (fp8 pair-tiles).  sA is never
            # read at k=0 (step-0 layer 1 is tower-only), so no
            # initialization is needed. ---
            sA = [[sp.tile([128, 2, RB], f8, tag=f"sA{j}_{r}",
                           name=f"sA{j}_{r}") for r in range(NR)]
                  for j in range(KT)]
            sB = [[sp.tile([128, 2, RB], f8, tag=f"sB{j}_{r}",
                           name=f"sB{j}_{r}") for r in range(NR)]
                  for j in range(KT)]

            # --- product accumulators ---
            pacc = []
            for r in range(NR):
                t = smp.tile([1, RB], f32, tag=f"pacc{r}", name=f"pacc{r}")
                nc.vector.memset(t, 1.0)
                pacc.append(t)

            # bias column: layer l in {0:Mb1 1:Mb2 2:Mb3 3:Ob1 4:Ob2} at
            # l*8+m; step-0 fused biases (b + agg@W1s) at 40+m (M), 48+m (O)
            def relu_epilogue(ot, ps, bias_col, m):
                """Bias+relu out of PSUM; alternate ACT/DVE by m so neither
                engine head-of-line-blocks the PE's psum bank rotation."""
                bias = ball[:, ds(bias_col, 1)]
                if m % 2 == 0:
                    nc.scalar.activation(ot, ps[:], Relu, bias=bias)
                else:
                    nc.vector.tensor_scalar(ot, ps[:], bias, 0.0, Add,
                                            mybir.AluOpType.max)

            def pair_tiles(tag):
                return [[ap.tile([128, 2, RB], f8, tag=tag, name=tag)
                         for _ in range(NR)] for _ in range(KT)]

            def l1_fused(scur, tow_t, include_m=True):
                """Fused M/O layer 1.  Per output tile m: all DoubleRow
                summary matmuls (stationary reused across the NR row
                blocks), then the contraction-64 tower closers with M on
                PE rows 0-63 and O on rows 64-127, issued adjacently so
                each M/O pair streams concurrently."""
                m1p = pair_tiles("m1p") if include_m else None
                h1p = pair_tiles("h1p")
                for m in range(FT):
                    psm = [pp.tile([128, RB], f32, tag="ps", name="psm")
                           for _ in range(NR)] if include_m else None
                    pso = [pp.tile([128, RB], f32, tag="ps", name="pso")
                           for _ in range(NR)]
                    for j in range(KT):
                        if include_m:
                            for r in range(NR):
                                nc.tensor.matmul(
                                    psm[r][:], mw1s[j][:, :, ts(m, 128)],
                                    scur[j][r][:], start=(j == 0),
                                    stop=False, perf_mode=DR)
                        for r in range(NR):
                            nc.tensor.matmul(
                                pso[r][:], ow1s[j][:, :, ts(m, 128)],
                                scur[j][r][:], start=(j == 0),
                                stop=False, perf_mode=DR)
                    for r in range(NR):
                        if include_m:
                            nc.tensor.matmul(
                                psm[r][:], mw1t[:, ts(m, 128)],
                                tow_t[0:NI, ts(r, RB)],
                                start=False, stop=True)
                        nc.tensor.matmul(
                            pso[r][:], ow1t[64:128, ts(m, 128)],
                            tow_t[64:128, ts(r, RB)],
                            start=False, stop=True)
                    q, i = divmod(m, 2)
                    for r in range(NR):
                        if include_m:
                            relu_epilogue(m1p[q][r][:, i, :], psm[r], m, m)
                        relu_epilogue(h1p[q][r][:, i, :], pso[r], 24 + m, m)
                return m1p, h1p

            def l1_k0(tow_t):
                """Step 0: summary is the broadcast aggregate, folded into
                the bias on the host, so layer 1 is just the tower matmul."""
                m1p = pair_tiles("m1p")
                h1p = pair_tiles("h1p")
                for m in range(FT):
                    psm = [pp.tile([128, RB], f32, tag="ps", name="psm")
                           for _ in range(NR)]
                    pso = [pp.tile([128, RB], f32, tag="ps", name="pso")
                           for _ in range(NR)]
                    for r in range(NR):
                        nc.tensor.matmul(
                            psm[r][:], mw1t[:, ts(m, 128)],
                            tow_t[0:NI, ts(r, RB)], start=True, stop=True)
                        nc.tensor.matmul(
                            pso[r][:], ow1t[64:128, ts(m, 128)],
                            tow_t[64:128, ts(r, RB)], start=True, stop=True)
                    q, i = divmod(m, 2)
                    for r in range(NR):
                        relu_epilogue(m1p[q][r][:, i, :], psm[r], 40 + m, m)
                        relu_epilogue(h1p[q][r][:, i, :], pso[r], 48 + m, m)
                return m1p, h1p

            def layer_dr(rhs, ws, writer):
                """1024x1024 DoubleRow layer: per output tile m, KT
                contraction matmuls x NR row blocks, stationary reused
                across row blocks."""
                for m in range(FT):
                    pss = [pp.tile([128, RB], f32, tag="ps", name="ps")
                           for _ in range(NR)]
                    for j in range(KT):
                        for r in range(NR):
                            nc.tensor.matmul(
                                pss[r][:], ws[j][:, :, ts(m, 128)],
                                rhs[j][r][:], start=(j == 0),
                                stop=(j == KT - 1), perf_mode=DR)
                    for r in range(NR):
                        writer(m, r, pss[r])

            def flush_zjobs(zjobs):
                for gb, r in zjobs:
                    zps = zp.tile([1, RB], f32, tag="z", name="zps")
                    nc.tensor.matmul(zps[:], ones[:], gb[:],
                                     start=True, stop=True)
                    pr = smp.tile([1, RB], f32, tag=f"pr{r}", name=f"pr{r}")
                    nc.scalar.activation(pr[:], zps[:], Sigmoid, bias=ob3[:])
                    nc.vector.tensor_mul(pacc[r][:], pacc[r][:], pr[:])

            scur, snxt = sA, sB
            zjobs = []
            for k in range(K):
                tow_t = twp.tile([128, R], bf, tag="tow", name="tow")
                nc.gpsimd.dma_start(out=tow_t[0:NI, :], in_=towd[k])
                nc.gpsimd.dma_start(out=tow_t[64:128, :], in_=towd[k])

                if k == 0:
                    m1, h1 = l1_k0(tow_t)
                elif k == K - 1:
                    # the final scan carry is discarded by the reference, so
                    # the last step's M branch (M1/M2/M3) is dead code
                    _, h1 = l1_fused(scur, tow_t, include_m=False)
                    m1 = None
                else:
                    m1, h1 = l1_fused(scur, tow_t)

                if k < K - 1:
                    m2p = pair_tiles("m2p")

                    def w_m2(m, r, ps):
                        q, i = divmod(m, 2)
                        relu_epilogue(m2p[q][r][:, i, :], ps, 8 + m, m)

                    layer_dr(m1, mw2, w_m2)
                    # previous step's output head (its DVE reduce is long
                    # done, so the sigmoid never head-of-line-blocks the
                    # ACT queue)
                    flush_zjobs(zjobs)
                    zjobs = []

                    def w_m3(m, r, ps):
                        q, i = divmod(m, 2)
                        relu_epilogue(snxt[q][r][:, i, :], ps, 16 + m, m)

                    layer_dr(m2p, mw3, w_m3)
                else:
                    flush_zjobs(zjobs)
                    zjobs = []

                h2 = [[None] * NR for _ in range(FT)]

                def w_o2(m, r, ps):
                    t = ap.tile([128, RB], bf, tag="h2", name="h2")
                    relu_epilogue(t[:], ps, 32 + m, m)
                    h2[m][r] = t

                layer_dr(h1, ow2, w_o2)

                # g = sum_i h2_i * w3_i on the DVE (per-partition scalars),
                # reduced across partitions next step by a ones-matmul.
                for r in range(NR):
                    if k < K - 1:
                        # DVE-serial chain; latency is hidden by the next
                        # step's PE work
                        g = zw.tile([128, RB], f32, tag="g", name="g")
                        nc.vector.tensor_scalar(
                            g[:], h2[0][r][:], w3c[:, ds(0, 1)], None, Mult)
                        for i in range(1, FT):
                            t = zw.tile([128, RB], f32, tag="t", name="t",
                                        bufs=3)
                            nc.vector.tensor_scalar(
                                t[:], h2[i][r][:], w3c[:, ds(i, 1)], None,
                                Mult)
                            nc.vector.tensor_tensor(g[:], g[:], t[:], Add)
                    else:
                        # final step: the chain is on the kernel's critical
                        # tail, so pipeline the multiplies on the otherwise
                        # idle scalar engine against the DVE adds
                        tts = []
                        g = None
                        for i in range(FT):
                            t = zw.tile([128, RB], f32, tag="t", name="t",
                                        bufs=3)
                            nc.scalar.activation(t[:], h2[i][r][:],
                                                 Identity,
                                                 scale=w3c[:, ds(i, 1)])
                            tts.append(t)
                            if i == 1:
                                g = zw.tile([128, RB], f32, tag="g",
                                            name="g")
                                nc.vector.tensor_tensor(
                                    g[:], tts[0][:], tts[1][:], Add)
                            elif i >= 2:
                                nc.vector.tensor_tensor(g[:], g[:], t[:],
                                                        Add)
                    gb = zw.tile([128, RB], bf, tag="gb", name="gb", bufs=4)
                    nc.vector.tensor_copy(gb[:], g[:])
                    zjobs.append((gb, r))

                scur, snxt = snxt, scur
            flush_zjobs(zjobs)

            for r in range(NR):
                nc.sync.dma_start(out=outd[:, ts(r, RB)], in_=pacc[r][:])

    nc.finalize()
    return nc


def _get_nc():
    global _BUILT
    if _BUILT is None:
        _BUILT = _build()
    return _BUILT


def _dr_quant(W):
    """[NH, NH] f32 -> [KT, 128, 2, NH] e4m3 DoubleRow interleave:
    out[j, p, i, m] = W[256j + 128i + p, m]."""
    return np.ascontiguousarray(
        W.reshape(KT, 2, 128, NH).transpose(0, 2, 1, 3)).astype(F8)


def _prep_inputs(inputs):
    f32 = np.float32
    towers = np.asarray(inputs["towers"], dtype=f32)
    agg = np.asarray(inputs["aggregate"], dtype=f32)
    MW1 = np.asarray(inputs["MW1"], dtype=f32)
    OW1 = np.asarray(inputs["OW1"], dtype=f32)

    def col8(v):
        return np.asarray(v, f32).reshape(FT, 128).T

    # step-0 biases with the broadcast-aggregate layer-1 contribution folded
    b0m = np.asarray(inputs["Mb1"], f32) + agg[0] @ MW1[:NH]
    b0o = np.asarray(inputs["Ob1"], f32) + agg[0] @ OW1[:NH]

    shared = {
        "mw1s": _dr_quant(MW1[:NH]),
        "mw1t": np.ascontiguousarray(MW1[NH:]).astype(BF16),
        "mw2": _dr_quant(np.asarray(inputs["MW2"], f32)),
        "mw3": _dr_quant(np.asarray(inputs["MW3"], f32)),
        "ow1s": _dr_quant(OW1[:NH]),
        "ow1t": np.ascontiguousarray(OW1[NH:]).astype(BF16),
        "ow2": _dr_quant(np.asarray(inputs["OW2"], f32)),
        "w3c": np.ascontiguousarray(col8(np.asarray(inputs["OW3"], f32))),
        "ball": np.ascontiguousarray(np.concatenate(
            [col8(inputs[b]) for b in ("Mb1", "Mb2", "Mb3", "Ob1", "Ob2")]
            + [col8(b0m), col8(b0o)], axis=1)),
        "ob3": np.asarray(inputs["Ob3"], f32).reshape(1, 1),
    }
    in_maps = []
    for c in range(N_CORES):
        tc_ = towers[c * R:(c + 1) * R]          # (R, K, NI)
        towT = np.ascontiguousarray(tc_.transpose(1, 2, 0)).astype(BF16)
        in_maps.append({"tow": towT, **shared})
    return in_maps


def _run(inputs, trace=False):
    nc = _get_nc()
    in_maps = _prep_inputs(inputs)
    res = run_bass_kernel_spmd(nc, in_maps, list(range(N_CORES)), trace=trace)
    out = np.concatenate([res.results[c]["out"][0] for c in range(N_CORES)])
    return out.astype(np.float32), res


def kernel(**inputs):
    out, _ = _run(inputs, trace=False)
    return out


# revision 9
# speedup vs baseline: 1.8847x; 1.0301x over previous
"""Trainium2 Bass kernel for nn_BottomUpNet (dense_mlp).

Reference computation (per row n of N=8192, fully independent across rows):
    summary = aggregate (broadcast)                   # (1024,)
    for k in 0..15:
        x = [summary, towers[n, k, :]]                # (1088,)
        h = relu(x @ OW1 + Ob1); h = relu(h @ OW2 + Ob2)
        pred_k = sigmoid(h @ OW3 + Ob3)
        m = relu(x @ MW1 + Mb1); m = relu(m @ MW2 + Mb2); m = relu(m @ MW3 + Mb3)
        summary = m
    out[n] = prod_k pred_k

Strategy: data-parallel over N across 8 cores (1024 rows each), weights
replicated.  Activations are feature-major ([feature partition, row free])
so weight matrices serve directly as the stationary matmul operand and no
on-chip transposes are needed.

The five 1024-contraction matmuls per step (M1s, M2, M3, O1s, O2) run in
fp8-e4m3 with perf_mode=DoubleRow: both operands carry contraction pairs
[128, 2, free] so each matmul instruction reduces 256 rows (2 fp8 weights
per PE cell), ~1.7x the bf16 streaming rate.  Weights are pre-interleaved
on the host into [ktile, 128, 2, NH]; activations feeding these matmuls
are written by the epilogues as fp8 pair-tiles [128, 2, 512].  End-to-end
rel err vs the f32 reference ~7e-3 (fp8 quantization noise; the e4m3
denormal range covers the small uniform weights acceptably, so no weight
scaling is needed and bias+relu epilogues keep their single-op form).
f32 PSUM accumulation throughout; the 64-wide tower matmuls and the
output head stay bf16/f32.

Perf structure:
  - loop order is m-outer / row-block-inner so each DoubleRow stationary
    tile (256x128 weight block) is reused by NR=2 matmuls, halving
    LDWEIGHTS traffic (DoubleRow weight loads are 2x the columns).
  - layer-1 tower closers for the M- and O-branches are paired into
    disjoint PE row groups (0-63 / 64-127), issued adjacently so each
    M/O pair streams concurrently in the systolic array.
  - step 0's summary is the broadcast aggregate, identical for all rows:
    its layer-1 contribution agg @ W1s is folded into the step-0 bias on
    the host, so step 0's layer 1 is just the tower matmul.
  - the 1024->1 output head is a DVE per-partition multiply/add tree
    (g = sum_i h2_i * w3_i) plus a single ones-vector matmul for the
    cross-partition reduce; its sigmoid + product-accumulate are deferred
    into the next step so they never head-of-line-block the scalar queue.
  - the final step's M branch (M1/M2/M3) is skipped entirely: the
    reference discards the last scan carry, so that summary is dead.
  - relu epilogues alternate between the scalar and vector engines;
    weight DMAs are split across the DGE queues strictly in first-use
    order (step 0 needs only the tower weights + biases to start).
"""

import numpy as np
import ml_dtypes

import concourse.bacc as bacc
import concourse.mybir as mybir
import concourse.tile as tile
from concourse.bass import ts, ds
from concourse.bass_utils import run_bass_kernel_spmd

BF16 = ml_dtypes.bfloat16
F8 = ml_dtypes.float8_e4m3

N_CORES = 8
N = 8192
K = 16
NI = 64          # tower features per step
NH = 1024        # hidden width
FT = NH // 128   # feature tiles (8)
KT = NH // 256   # DoubleRow contraction tiles (4)
R = N // N_CORES  # rows per core (1024)
RB = 512         # row block (matmul moving dim / one PSUM bank)
NR = R // RB     # row blocks per core (2)

_BUILT = None


def _build():
    nc = bacc.Bacc("TRN2", target_bir_lowering=False, debug=False,
                   num_devices=N_CORES)
    f32 = mybir.dt.float32
    bf = mybir.dt.bfloat16
    f8 = mybir.dt.float8e4
    DR = mybir.MatmulPerfMode.DoubleRow

    towd = nc.declare_dram_parameter("tow", [K, NI, R], bf, isOutput=False)
    mw1sd = nc.declare_dram_parameter("mw1s", [KT, 128, 2, NH], f8,
                                      isOutput=False)
    mw1td = nc.declare_dram_parameter("mw1t", [NI, NH], bf, isOutput=False)
    mw2d = nc.declare_dram_parameter("mw2", [KT, 128, 2, NH], f8,
                                     isOutput=False)
    mw3d = nc.declare_dram_parameter("mw3", [KT, 128, 2, NH], f8,
                                     isOutput=False)
    ow1sd = nc.declare_dram_parameter("ow1s", [KT, 128, 2, NH], f8,
                                      isOutput=False)
    ow1td = nc.declare_dram_parameter("ow1t", [NI, NH], bf, isOutput=False)
    ow2d = nc.declare_dram_parameter("ow2", [KT, 128, 2, NH], f8,
                                     isOutput=False)
    w3cd = nc.declare_dram_parameter("w3c", [128, FT], f32, isOutput=False)
    balld = nc.declare_dram_parameter("ball", [128, 56], f32, isOutput=False)
    ob3d = nc.declare_dram_parameter("ob3", [1, 1], f32, isOutput=False)
    outd = nc.declare_dram_parameter("out", [1, R], f32, isOutput=True)

    Relu = mybir.ActivationFunctionType.Relu
    Sigmoid = mybir.ActivationFunctionType.Sigmoid
    Identity = mybir.ActivationFunctionType.Identity
    Add = mybir.AluOpType.add
    Mult = mybir.AluOpType.mult

    with tile.TileContext(nc) as tc:
        with (
            tc.tile_pool(name="weights", bufs=1) as wp,
            tc.tile_pool(name="summary", bufs=1) as sp,
            tc.tile_pool(name="acts", bufs=16) as ap,
            tc.tile_pool(name="tow", bufs=4) as twp,
            tc.tile_pool(name="small", bufs=1) as smp,
            tc.tile_pool(name="zwork", bufs=2) as zw,
            tc.tile_pool(name="psum", bufs=6, space="PSUM") as pp,
            tc.tile_pool(name="zpsum", bufs=2, space="PSUM") as zp,
        ):
            # --- small/early tiles on the gpsimd SW queue; step 0 only
            # needs the tower weights + biases to start.  The step-0 tower
            # weights + tower data go first on the two HW DGE queues so the
            # PE can start ~10us sooner. ---
            ball = smp.tile([128, 56], f32, tag="ball", name="ball")
            nc.gpsimd.dma_start(out=ball, in_=balld[:])
            ob3 = smp.tile([1, 1], f32, tag="ob3", name="ob3")
            nc.gpsimd.dma_start(out=ob3, in_=ob3d[:])
            w3c = smp.tile([128, FT], f32, tag="w3c", name="w3c")
            nc.gpsimd.dma_start(out=w3c, in_=w3cd[:])
            mw1t = wp.tile([NI, NH], bf, tag="mw1t", name="mw1t")
            nc.sync.dma_start(out=mw1t, in_=mw1td[:])
            ow1t = wp.tile([128, NH], bf, tag="ow1t", name="ow1t")
            nc.scalar.dma_start(out=ow1t[64:128, :], in_=ow1td[:])
            tow0 = twp.tile([128, R], bf, tag="tow", name="tow")
            nc.sync.dma_start(out=tow0[0:NI, :], in_=towd[0])
            nc.scalar.dma_start(out=tow0[64:128, :], in_=towd[0])

            # --- DoubleRow weights on the two HW DGE queues, strictly in
            # first-use order (step 0: M2, M3, O2; step 1 adds M1s, O1s) ---
            _q = [0]

            def load_dr(dram, name):
                tiles = []
                for j in range(KT):
                    t = wp.tile([128, 2, NH], f8, tag=f"{name}{j}",
                                name=f"{name}{j}")
                    eng = (nc.sync, nc.scalar)[_q[0] % 2]
                    _q[0] += 1
                    eng.dma_start(out=t, in_=dram[j])
                    tiles.append(t)
                return tiles

            mw2 = load_dr(mw2d, "mw2")
            mw3 = load_dr(mw3d, "mw3")
            ow2 = load_dr(ow2d, "ow2")
            mw1s = load_dr(mw1sd, "mw1s")
            ow1s = load_dr(ow1sd, "ow1s")

            ones = smp.tile([128, 1], bf, tag="ones", name="ones")
            nc.vector.memset(ones, 1.0)

            # --- summary double buffer (fp8 pair-tiles).  sA is never
            # read at k=0 (step-0 layer 1 is tower-only), so no
            # initialization is needed. ---
            sA = [[sp.tile([128, 2, RB], f8, tag=f"sA{j}_{r}",
                           name=f"sA{j}_{r}") for r in range(NR)]
                  for j in range(KT)]
            sB = [[sp.tile([128, 2, RB], f8, tag=f"sB{j}_{r}",
                           name=f"sB{j}_{r}") for r in range(NR)]
                  for j in range(KT)]

            # --- product accumulators ---
            pacc = []
            for r in range(NR):
                t = smp.tile([1, RB], f32, tag=f"pacc{r}", name=f"pacc{r}")
                nc.vector.memset(t, 1.0)
                pacc.append(t)

            # bias column: layer l in {0:Mb1 1:Mb2 2:Mb3 3:Ob1 4:Ob2} at
            # l*8+m; step-0 fused biases (b + agg@W1s) at 40+m (M), 48+m (O)
            def relu_epilogue(ot, ps, bias_col, m):
                """Bias+relu out of PSUM; alternate ACT/DVE by m so neither
                engine head-of-line-blocks the PE's psum bank rotation."""
                bias = ball[:, ds(bias_col, 1)]
                if m % 2 == 0:
                    nc.scalar.activation(ot, ps[:], Relu, bias=bias)
                else:
                    nc.vector.tensor_scalar(ot, ps[:], bias, 0.0, Add,
                                            mybir.AluOpType.max)

            def pair_tiles(tag):
                return [[ap.tile([128, 2, RB], f8, tag=tag, name=tag)
                         for _ in range(NR)] for _ in range(KT)]

            def l1_fused(scur, tow_t, include_m=True, deferred=()):
                """Fused M/O layer 1.  Per output tile m: all DoubleRow
                summary matmuls (stationary reused across the NR row
                blocks), then the contraction-64 tower closers with M on
                PE rows 0-63 and O on rows 64-127, issued adjacently so
                each M/O pair streams concurrently.  `deferred` is a list
                of closures (the previous step's DVE head-chain ops),
                drained ~4 per m-iteration so they interleave with this
                layer's epilogues in the DVE FIFO instead of forming a
                9us burst that blocks PSUM bank release."""
                m1p = pair_tiles("m1p") if include_m else None
                h1p = pair_tiles("h1p")
                dq = list(deferred)
                for m in range(FT):
                    psm = [pp.tile([128, RB], f32, tag="ps", name="psm")
                           for _ in range(NR)] if include_m else None
                    pso = [pp.tile([128, RB], f32, tag="ps", name="pso")
                           for _ in range(NR)]
                    for j in range(KT):
                        if include_m:
                            for r in range(NR):
                                nc.tensor.matmul(
                                    psm[r][:], mw1s[j][:, :, ts(m, 128)],
                                    scur[j][r][:], start=(j == 0),
                                    stop=False, perf_mode=DR)
                        for r in range(NR):
                            nc.tensor.matmul(
                                pso[r][:], ow1s[j][:, :, ts(m, 128)],
                                scur[j][r][:], start=(j == 0),
                                stop=False, perf_mode=DR)
                    for r in range(NR):
                        if include_m:
                            nc.tensor.matmul(
                                psm[r][:], mw1t[:, ts(m, 128)],
                                tow_t[0:NI, ts(r, RB)],
                                start=False, stop=True)
                        nc.tensor.matmul(
                            pso[r][:], ow1t[64:128, ts(m, 128)],
                            tow_t[64:128, ts(r, RB)],
                            start=False, stop=True)
                    q, i = divmod(m, 2)
                    for r in range(NR):
                        if include_m:
                            relu_epilogue(m1p[q][r][:, i, :], psm[r], m, m)
                        relu_epilogue(h1p[q][r][:, i, :], pso[r], 24 + m,
                                      m + 1)
                    take = (len(dq) + FT - 1 - m) // (FT - m)
                    for _ in range(take):
                        dq.pop(0)()
                return m1p, h1p

            def l1_k0(tow_t):
                """Step 0: summary is the broadcast aggregate, folded into
                the bias on the host, so layer 1 is just the tower matmul."""
                m1p = pair_tiles("m1p")
                h1p = pair_tiles("h1p")
                for m in range(FT):
                    psm = [pp.tile([128, RB], f32, tag="ps", name="psm")
                           for _ in range(NR)]
                    pso = [pp.tile([128, RB], f32, tag="ps", name="pso")
                           for _ in range(NR)]
                    for r in range(NR):
                        nc.tensor.matmul(
                            psm[r][:], mw1t[:, ts(m, 128)],
                            tow_t[0:NI, ts(r, RB)], start=True, stop=True)
                        nc.tensor.matmul(
                            pso[r][:], ow1t[64:128, ts(m, 128)],
                            tow_t[64:128, ts(r, RB)], start=True, stop=True)
                    q, i = divmod(m, 2)
                    for r in range(NR):
                        relu_epilogue(m1p[q][r][:, i, :], psm[r], 40 + m, m)
                        relu_epilogue(h1p[q][r][:, i, :], pso[r], 48 + m,
                                      m + 1)
                return m1p, h1p

            def layer_dr(rhs, ws, writer):
                """1024x1024 DoubleRow layer: per output tile m, KT
                contraction matmuls x NR row blocks, stationary reused
                across row blocks."""
                for m in range(FT):
                    pss = [pp.tile([128, RB], f32, tag="ps", name="ps")
                           for _ in range(NR)]
                    for j in range(KT):
                        for r in range(NR):
                            nc.tensor.matmul(
                                pss[r][:], ws[j][:, :, ts(m, 128)],
                                rhs[j][r][:], start=(j == 0),
                                stop=(j == KT - 1), perf_mode=DR)
                    for r in range(NR):
                        writer(m, r, pss[r])

            def flush_zjobs(zjobs):
                for gb, r in zjobs:
                    zps = zp.tile([1, RB], f32, tag="z", name="zps")
                    nc.tensor.matmul(zps[:], ones[:], gb[:],
                                     start=True, stop=True)
                    pr = smp.tile([1, RB], f32, tag=f"pr{r}", name=f"pr{r}")
                    nc.scalar.activation(pr[:], zps[:], Sigmoid, bias=ob3[:])
                    nc.vector.tensor_mul(pacc[r][:], pacc[r][:], pr[:])

            def head_chain_ops(h2, r, zjobs):
                """The k<K-1 output head as a list of closures: mult/add
                chain on the DVE, drained interleaved into the next step's
                layer-1 loop."""
                st = {}
                ops = []

                def op0():
                    g = zw.tile([128, RB], f32, tag="g", name="g")
                    nc.vector.tensor_scalar(
                        g[:], h2[0][r][:], w3c[:, ds(0, 1)], None, Mult)
                    st["g"] = g
                ops.append(op0)
                for i in range(1, FT):
                    def opm(i=i):
                        t = zw.tile([128, RB], f32, tag="t", name="t",
                                    bufs=3)
                        nc.vector.tensor_scalar(
                            t[:], h2[i][r][:], w3c[:, ds(i, 1)], None, Mult)
                        st["t"] = t

                    def opa():
                        nc.vector.tensor_tensor(st["g"][:], st["g"][:],
                                                st["t"][:], Add)
                    ops += [opm, opa]

                def opc():
                    gb = zw.tile([128, RB], bf, tag="gb", name="gb", bufs=4)
                    nc.vector.tensor_copy(gb[:], st["g"][:])
                    zjobs.append((gb, r))
                ops.append(opc)
                return ops

            def o2_final(h1, zjobs):
                """Final-step O2 + output head, row-block-outer so r=0's
                head chain drains while r=1's matmuls stream; ACT does the
                w3 multiplies straight off each h2 epilogue, the DVE adds
                form a binary tree, so the post-matmul tail is ~4us instead
                of a 13us serialized two-engine ping-pong."""
                for r in range(NR):
                    tt = [None] * FT
                    u = [None] * (FT // 2)

                    def add(a, b):
                        o = zw.tile([128, RB], f32, tag="u", name="u",
                                    bufs=4)
                        nc.vector.tensor_tensor(o[:], a[:], b[:], Add)
                        return o

                    for m in range(FT):
                        ps = pp.tile([128, RB], f32, tag="ps", name="ps")
                        for j in range(KT):
                            nc.tensor.matmul(
                                ps[:], ow2[j][:, :, ts(m, 128)],
                                h1[j][r][:], start=(j == 0),
                                stop=(j == KT - 1), perf_mode=DR)
                        h2t = ap.tile([128, RB], bf, tag="h2", name="h2")
                        relu_epilogue(h2t[:], ps, 32 + m, m)
                        t = zw.tile([128, RB], f32, tag="t", name="t",
                                    bufs=3)
                        nc.scalar.activation(t[:], h2t[:], Identity,
                                             scale=w3c[:, ds(m, 1)])
                        tt[m] = t
                        if m % 2 == 1:
                            u[m // 2] = add(tt[m - 1], tt[m])
                        if m == 3:
                            u[0] = add(u[0], u[1])
                        if m == 7:
                            u[2] = add(u[2], u[3])
                            g = add(u[0], u[2])
                            gb = zw.tile([128, RB], bf, tag="gb", name="gb",
                                         bufs=4)
                            nc.vector.tensor_copy(gb[:], g[:])
                            zjobs.append((gb, r))

            scur, snxt = sA, sB
            zjobs = []
            pending = []
            for k in range(K):
                if k == 0:
                    tow_t = tow0
                else:
                    tow_t = twp.tile([128, R], bf, tag="tow", name="tow")
                    nc.gpsimd.dma_start(out=tow_t[0:NI, :], in_=towd[k])
                    nc.gpsimd.dma_start(out=tow_t[64:128, :], in_=towd[k])

                if k == 0:
                    m1, h1 = l1_k0(tow_t)
                elif k == K - 1:
                    # the final scan carry is discarded by the reference, so
                    # the last step's M branch (M1/M2/M3) is dead code
                    _, h1 = l1_fused(scur, tow_t, include_m=False,
                                     deferred=pending)
                    pending = []
                    m1 = None
                else:
                    m1, h1 = l1_fused(scur, tow_t, deferred=pending)
                    pending = []

                if k < K - 1:
                    m2p = pair_tiles("m2p")

                    def w_m2(m, r, ps):
                        q, i = divmod(m, 2)
                        relu_epilogue(m2p[q][r][:, i, :], ps, 8 + m, m)

                    layer_dr(m1, mw2, w_m2)
                    # previous step's output head (its DVE reduce is long
                    # done, so the sigmoid never head-of-line-blocks the
                    # ACT queue)
                    flush_zjobs(zjobs)
                    zjobs = []

                    def w_m3(m, r, ps):
                        q, i = divmod(m, 2)
                        relu_epilogue(snxt[q][r][:, i, :], ps, 16 + m, m)

                    layer_dr(m2p, mw3, w_m3)
                else:
                    flush_zjobs(zjobs)
                    zjobs = []

                if k < K - 1:
                    h2 = [[None] * NR for _ in range(FT)]

                    def w_o2(m, r, ps):
                        t = ap.tile([128, RB], bf, tag="h2", name="h2")
                        relu_epilogue(t[:], ps, 32 + m, m)
                        h2[m][r] = t

                    layer_dr(h1, ow2, w_o2)
                    # g = sum_i h2_i * w3_i on the DVE (per-partition
                    # scalars), reduced across partitions next step by a
                    # ones-matmul.  Emission is deferred into the next
                    # step's layer-1 loop (chains for r=0/r=1 interleaved).
                    c0 = head_chain_ops(h2, 0, zjobs)
                    c1 = head_chain_ops(h2, 1, zjobs)
                    pending = [op for pair in zip(c0, c1) for op in pair]
                else:
                    o2_final(h1, zjobs)

                scur, snxt = snxt, scur
            flush_zjobs(zjobs)

            for r in range(NR):
                nc.sync.dma_start(out=outd[:, ts(r, RB)], in_=pacc[r][:])

    nc.finalize()
    return nc


def _get_nc():
    global _BUILT
    if _BUILT is None:
        _BUILT = _build()
    return _BUILT


def _dr_quant(W):
    """[NH, NH] f32 -> [KT, 128, 2, NH] e4m3 DoubleRow interleave:
    out[j, p, i, m] = W[256j + 128i + p, m]."""
    return np.ascontiguousarray(
        W.reshape(KT, 2, 128, NH).transpose(0, 2, 1, 3)).astype(F8)


def _prep_inputs(inputs):
    f32 = np.float32
    towers = np.asarray(inputs["towers"], dtype=f32)
    agg = np.asarray(inputs["aggregate"], dtype=f32)
    MW1 = np.asarray(inputs["MW1"], dtype=f32)
    OW1 = np.asarray(inputs["OW1"], dtype=f32)

    def col8(v):
        return np.asarray(v, f32).reshape(FT, 128).T

    # step-0 biases with the broadcast-aggregate layer-1 contribution folded
    b0m = np.asarray(inputs["Mb1"], f32) + agg[0] @ MW1[:NH]
    b0o = np.asarray(inputs["Ob1"], f32) + agg[0] @ OW1[:NH]

    shared = {
        "mw1s": _dr_quant(MW1[:NH]),
        "mw1t": np.ascontiguousarray(MW1[NH:]).astype(BF16),
        "mw2": _dr_quant(np.asarray(inputs["MW2"], f32)),
        "mw3": _dr_quant(np.asarray(inputs["MW3"], f32)),
        "ow1s": _dr_quant(OW1[:NH]),
        "ow1t": np.ascontiguousarray(OW1[NH:]).astype(BF16),
        "ow2": _dr_quant(np.asarray(inputs["OW2"], f32)),
        "w3c": np.ascontiguousarray(col8(np.asarray(inputs["OW3"], f32))),
        "ball": np.ascontiguousarray(np.concatenate(
            [col8(inputs[b]) for b in ("Mb1", "Mb2", "Mb3", "Ob1", "Ob2")]
            + [col8(b0m), col8(b0o)], axis=1)),
        "ob3": np.asarray(inputs["Ob3"], f32).reshape(1, 1),
    }
    in_maps = []
    for c in range(N_CORES):
        tc_ = towers[c * R:(c + 1) * R]          # (R, K, NI)
        towT = np.ascontiguousarray(tc_.transpose(1, 2, 0)).astype(BF16)
        in_maps.append({"tow": towT, **shared})
    return in_maps


def _run(inputs, trace=False):
    nc = _get_nc()
    in_maps = _prep_inputs(inputs)
    res = run_bass_kernel_spmd(nc, in_maps, list(range(N_CORES)), trace=trace)
    out = np.concatenate([res.results[c]["out"][0] for c in range(N_CORES)])
    return out.astype(np.float32), res


def kernel(**inputs):
    out, _ = _run(inputs, trace=False)
    return out
